# revision 27
# baseline (speedup 1.0000x reference)
"""Trainium2 kernel for per-class conditional dense (MoE-style routing).

    out[b] = x[b] @ W[classes[b]] + bias[classes[b]]
    x: [2048, 512] f32, classes: [2048, 1] int, W: [100, 512, 512] f32,
    bias: [100, 512] f32 -> out: [2048, 512] f32

Sharding: expert-parallel across 8 NeuronCores (grouped-GEMM style).
Class c is owned by core c // 13 (13 class slots per core). The host
routes each sample to the core owning its class, packing the samples of
each class into a fixed-width (S columns, zero-padded) block of a
transposed activation panel.

Precision: the weight table is the dominant HBM traffic, so it is
stored as TRN fp8 E3M4 (float8e3, 4 mantissa bits) at 1 byte/elem --
half the bf16 bytes. W is pre-scaled by a power of two s so its range
fills E3M4's [0.25, 15.5] normal band, and x is divided by the same s
(exact in bf16), so out = (x/s) @ (sW) = x @ W with no epilogue fixup.
The PE consumes the fp8 weights directly as the moving operand against
a bf16 stationary x panel (mixed-dtype matmul, 1 cycle/row). Measured
end-to-end relative error ~1.5e-2 (E3M4 quantization of W dominates),
inside the 2e-2 gate.

Data movement: ONE interleaved fp8 "mega" stream per core carries
everything in exact consumption order on a single Sync-HWDGE ring:
[x_slot0 | W_slot0 | x_slot1 | W_slot1 | ...], where each slot's bf16
x panel (transposed, k-tiled) rides as raw bytes inside the fp8 tensor
and is read back on-chip through a bitcast view. One ring = strict
FIFO arrival order (a slot's x always lands just before its weights),
2.3-6.9KB per-partition lines keep the 16 SDMA engines at their
per-packet service ceiling (~26 GB/s/engine), and no second ring
competes for packet slots during the PE-critical early phase.

The Tensor engine is the critical resource (~26.6K moving fp8 columns
= 11.1 us at 2.4 GHz): warm-up dummies bridge the DMA fill so the
DVFS p-state ramp (full clock only after ~3us of continuous matmuls,
reset by idle gaps) overlaps the wait; chunks ramp 0.5/0.5/1/1/2/3/3/
1/1 slots so the PE starts half a slot after first-byte and the stream
then stays just ahead; the final slot is split in U-halves on fresh
PSUM tiles so its first drain+store overlap its second half's matmuls
and the kernel-ending store is only 16 KB. PSUM groups (GRP slots
each) drain through Scalar-ACTIVATE / DVE alternately and store from
the Sync ring (whose FIFO defers store packets behind the remaining
weight stream for free). The host scatters the panel rows back to
sample order and adds the bias in fp32.
"""

import sys
import types

import numpy as np

try:
    import concourse.bass as bass
except ImportError:  # pragma: no cover - fallback for bare environments
    for _p in ("/opt/trn_rl_repo", "/root/.axon_site/_ro/trn_rl_repo"):
        if _p not in sys.path:
            sys.path.insert(0, _p)
    import concourse.bass as bass

try:  # pragma: no cover
    import antenv.axon_hooks  # noqa: F401
except ImportError:
    # bass_utils imports this when BASS_TRACE is set; the agent image's
    # antenv lacks it. Register a no-op shim so tracing degrades to a
    # plain (untraced) run instead of crashing.
    _hooks = types.ModuleType("antenv.axon_hooks")
    _hooks.get_axon_ntff_profile_hook = lambda: None
    _hooks.set_axon_ntff_profile_hook = lambda h: None
    sys.modules["antenv.axon_hooks"] = _hooks

import bass_rust
import ml_dtypes
import concourse.tile as tile
from concourse import mybir
from concourse.bass_utils import run_bass_kernel_spmd

B, D, U, C = 2048, 512, 512, 100
NCORES = 8
CPC = 13  # class slots per core (8 * 13 = 104 >= C)
PT = 128  # partition tile
KT = D // PT  # contraction-dim tiles
WSL = KT * U  # fp8 weight columns per slot
BF16 = ml_dtypes.bfloat16
FP8 = ml_dtypes.float8_e3m4
FP8_MAX = 15.5  # E3M4 max finite
N_WARM = 12  # 128-col PE warm-up matmuls (DVFS p-state ramp bridge)

_PROG_CACHE = {}
LAST_RESULTS = None  # BassKernelResults of the most recent device run


def _split_multi_waits(nc):
    """Walrus on this image only accepts one sync wait per instruction.

    Tile emits multi-wait instructions (notably the kernel-tail Drain,
    which waits on every live semaphore). Split each extra wait onto a
    same-engine NoOp inserted immediately before the instruction.
    """
    for fn in nc.m.functions:
        for bb in fn.blocks:
            new = []
            changed = False
            for inst in bb.instructions:
                si = inst.sync_info
                waits = list(si.on_wait) if si else []
                if len(waits) > 1:
                    for idx, w in enumerate(waits[:-1]):
                        nop = mybir.InstNoOp(
                            name=f"{inst.name}-waitsplit{idx}", ins=[], outs=[]
                        )
                        nop.engine = inst.engine
                        nop.sync_info = bass_rust.SyncInfo(
                            on_wait=[w], on_update=[]
                        )
                        new.append(nop)
                    inst.sync_info = bass_rust.SyncInfo(
                        on_wait=[waits[-1]], on_update=list(si.on_update)
                    )
                    changed = True
                new.append(inst)
            if changed:
                bb.instructions = new


def _build_program(S):
    """One SPMD program, shared by all 8 cores; per-core data differs.

    Per core: mega [PT, CPC*(XBLK+WSL)] float8e3 -- per slot, the bf16
    x panel block (as raw bytes) followed by the slot's pre-scaled E3M4
    weights -- -> out [NCOL, U] bf16.
    """
    f32 = mybir.dt.float32
    bf16 = mybir.dt.bfloat16
    fp8 = mybir.dt.float8e3
    NCOL = CPC * S
    GRP = PT // S  # class slots sharing one PSUM bank / output tile
    OG = -(-CPC // GRP)  # output groups
    BR = GRP * S  # rows per PSUM bank / output tile
    XBLK = 2 * KT * S  # fp8 columns of the slot's bf16 x panel block
    SLOT = XBLK + WSL  # fp8 columns per slot block
    HW = WSL // 2

    nc = bass.Bass()
    mega = nc.dram_tensor("mega", [PT, CPC * SLOT], fp8, kind="ExternalInput")
    out = nc.dram_tensor("out", [NCOL, U], bf16, kind="ExternalOutput")

    # Chunk column widths: slot 0 in halves (x + first two k-tiles of
    # W, then the rest) so the PE starts ASAP, ramp to 3-slot chunks
    # (6.9KB lines, full packet-service rate), taper at the end.
    chunks = [XBLK + HW, HW]
    for n in (1, 1, 2, 3, 3, 1, 1):
        chunks.append(n * SLOT)
    assert sum(chunks) == CPC * SLOT

    with tile.TileContext(nc) as tc:
        with (
            tc.tile_pool(name="mp", bufs=1) as mp,
            tc.tile_pool(name="op", bufs=1) as op,
            tc.tile_pool(name="pp", bufs=3, space="PSUM") as pp,
            tc.tile_pool(name="pb", bufs=2, space="PSUM") as pbpool,
            tc.tile_pool(name="ap", bufs=1, space="PSUM") as apool,
        ):
            m_t = mp.tile([PT, CPC * SLOT], fp8, name="m")
            scr_s = mp.tile([PT, PT], bf16, name="scr")
            scr_p = apool.tile([PT, 512], f32, name="scrp")

            col = 0
            for w in chunks:
                nc.sync.dma_start(m_t[:, col : col + w], mega[:, col : col + w])
                col += w

            # PE warm-up: the DVFS p-state needs ~3us of continuous
            # matmul execution to reach full clock, and idle gaps reset
            # it (measured: cold matmuls run at ~1.2GHz, half speed).
            # Fine-grained dummies on memset scratch bridge the DMA
            # fill; real matmuls continue the ramp seamlessly.
            nc.gpsimd.memset(scr_s[:], 1.0)
            for _ in range(N_WARM):
                nc.tensor.matmul(
                    scr_p[:, :PT],
                    scr_s[:],
                    scr_s[:],
                    start=True,
                    stop=True,
                    skip_group_check=True,
                )

            ots = [op.tile([BR, U], bf16, name=f"o{g}") for g in range(OG)]

            def stat_ap(j, i):
                # The slot's bf16 x panel, k-tile i: raw bytes live at
                # fp8 columns [j*SLOT + i*2S, +2S); bitcast back.
                base = j * SLOT + i * 2 * S
                return m_t[:, base : base + 2 * S].bitcast(bf16)

            def mov_ap(j, i, lo=0, hi=U):
                base = j * SLOT + XBLK + i * U
                return m_t[:, base + lo : base + hi]

            U2 = U // 2
            for j in range(CPC):
                g, r = divmod(j, GRP)
                last = j == CPC - 1 and r == 0
                rows = min(BR, NCOL - g * BR)
                if not last:
                    if r == 0:
                        ps = pp.tile([BR, U], f32, tag="ps", name=f"ps{g}")
                    for i in range(KT):
                        nc.tensor.matmul(
                            ps[S * r : S * r + S, :],
                            stat_ap(j, i),
                            mov_ap(j, i),
                            start=(i == 0),
                            stop=(i == KT - 1),
                            # PE-array column offset = PSUM partition
                            # offset; auto-infer rejects some offsets,
                            # so pass it explicitly.
                            tile_position=(0, S * r),
                        )
                else:
                    # Final slot split in U-halves on fresh PSUM tiles
                    # (dep tracking is partition-granular; a shared
                    # tile would serialize half B behind half A's
                    # drain): half A's drain and store overlap half B's
                    # matmuls, and the kernel-ending store shrinks to
                    # 16 KB.
                    for uo in (0, U2):
                        psh = pbpool.tile(
                            [S, U2], f32, tag="psh", name=f"h{uo}"
                        )
                        for i in range(KT):
                            nc.tensor.matmul(
                                psh[:, :],
                                stat_ap(j, i),
                                mov_ap(j, i, uo, uo + U2),
                                start=(i == 0),
                                stop=(i == KT - 1),
                                tile_position=(0, 0),
                            )
                        nc.vector.tensor_scalar_add(
                            ots[g][:rows, uo : uo + U2], psh[:rows, :], 0.0
                        )
                        q = nc.sync if uo == 0 else nc.scalar
                        q.dma_start(
                            out[g * BR : g * BR + rows, uo : uo + U2],
                            ots[g][:rows, uo : uo + U2],
                        )
                if j == 0:
                    # Fillers covering the gap between slot 0's matmuls
                    # and the arrival of the next chunk; they read the
                    # already-arrived slot-0 x block (no new waits).
                    for _ in range(3):
                        nc.tensor.matmul(
                            scr_p[:S, :S],
                            stat_ap(0, 0),
                            stat_ap(0, 0),
                            start=True,
                            stop=True,
                            skip_group_check=True,
                        )
                if (r == GRP - 1 or j == CPC - 1) and not last:
                    # Drains alternate Scalar-ACTIVATE / DVE so
                    # consecutive groups' drains overlap at the tail.
                    if g % 2 == 0:
                        nc.scalar.copy(ots[g][:rows, :], ps[:rows, :])
                    else:
                        nc.vector.tensor_scalar_add(
                            ots[g][:rows, :], ps[:rows, :], 0.0
                        )
                    # Early stores ride the Sync ring: its FIFO
                    # naturally defers their packets behind the
                    # remaining weight stream (no mid-stream bandwidth
                    # theft). The second-to-last group's trigger goes
                    # on Scalar so the tail triggers issue concurrently
                    # (each costs ~0.6us).
                    q = nc.scalar if g == OG - 2 else nc.sync
                    q.dma_start(
                        out[g * BR : g * BR + rows, :], ots[g][:rows, :]
                    )
    _split_multi_waits(nc)
    return nc


def kernel(x, classes, kernel, bias):
    global LAST_RESULTS
    x = np.asarray(x, dtype=np.float32)
    W = np.asarray(kernel, dtype=np.float32)
    bias_np = np.asarray(bias, dtype=np.float32)
    cls = np.asarray(classes).reshape(-1).astype(np.int64)

    counts = np.bincount(cls, minlength=C)
    # Fixed column width per class slot; multiple of 8 for DMA alignment.
    S = int(max(32, -(-counts.max() // 8) * 8))
    if S not in _PROG_CACHE:
        _PROG_CACHE[S] = _build_program(S)
    nc = _PROG_CACHE[S]
    NCOL = CPC * S
    XBLK = 2 * KT * S
    SLOT = XBLK + WSL

    # Power-of-two weight scale filling E3M4's normal band; x carries
    # the inverse scale exactly (exponent shift), so out = x @ W.
    absmax = float(np.abs(W).max())
    s = float(2.0 ** np.floor(np.log2(FP8_MAX / absmax))) if absmax > 0 else 1.0

    order = np.argsort(cls, kind="stable")
    starts = np.zeros(C + 1, np.int64)
    np.cumsum(counts[:C], out=starts[1:])
    rows_by_class = [order[starts[c] : starts[c + 1]] for c in range(C)]

    # Weight slots, pre-tiled to the SBUF layout and cast to E3M4:
    # [c, p, i*U+u] holds s*W[c, i*128+p, u].
    W_t8 = (
        (W * s)
        .reshape(C, KT, PT, U)
        .transpose(0, 2, 1, 3)
        .reshape(C, PT, KT * U)
        .astype(FP8)
    )

    xs = x * np.float32(1.0 / s)
    in_maps = []
    for m in range(NCORES):
        xt_m = np.zeros((D, NCOL), np.float32)
        for j in range(CPC):
            c = m * CPC + j
            if c >= C:
                continue
            r = rows_by_class[c]
            if r.size:
                xt_m[:, S * j : S * j + r.size] = xs[r].T
        # Pre-tile x panel: [p, i*NCOL + c] = xt[i*128+p, c].
        xt_dev = np.ascontiguousarray(
            xt_m.reshape(KT, PT, NCOL).transpose(1, 0, 2).reshape(PT, KT * NCOL)
        ).astype(BF16)
        # Interleaved mega stream: per slot, the bf16 x block (raw
        # bytes viewed as fp8) then the slot's weights.
        mega_m = np.empty((PT, CPC * SLOT), FP8)
        for j in range(CPC):
            c = (m * CPC + j) % C
            xb = np.ascontiguousarray(
                np.concatenate(
                    [
                        xt_dev[:, i * NCOL + S * j : i * NCOL + S * (j + 1)]
                        for i in range(KT)
                    ],
                    axis=1,
                )
            ).view(FP8)
            mega_m[:, j * SLOT : j * SLOT + XBLK] = xb
            mega_m[:, j * SLOT + XBLK : (j + 1) * SLOT] = W_t8[c]
        in_maps.append({"mega": mega_m})

    res = run_bass_kernel_spmd(nc, in_maps, list(range(NCORES)))
    LAST_RESULTS = res

    out = np.empty((B, U), np.float32)
    for m in range(NCORES):
        panel = np.asarray(res.results[m]["out"]).astype(np.float32)
        for j in range(CPC):
            c = m * CPC + j
            if c >= C:
                continue
            r = rows_by_class[c]
            if r.size:
                out[r] = panel[S * j : S * j + r.size] + bias_np[c]
    return out
